# revision 21
# baseline (speedup 1.0000x reference)
"""JPEG layer (nn_JpegLayer) Trainium2 Bass kernel, 8-core data parallel.

Pipeline per image (per core: 4 images of [3,512,512]):
  P1 : 3-accum matmuls fold RGB->YCC color mix + H-DCT; chroma additionally
       W-pools via paired stride-2 rhs matmuls (6 accums at N=256).  Y level
       shift (-sqrt8*L at DC rows) folds into the eviction bias (scalar eng).
  T1 : PE transposes -> [w, h'freq]
  P2 : W-DCT (0.5x pool fold for chroma) -> full 2D coeffs psum
  Q  : ey = d*(1/q) (DVE); round via +/-1.5*2^23 (DVE); dec = r*q -> bf16
  ITP: fused W-IDCT + transpose as plain matmuls with the dec block as
       stationary: psum[., s] = dec_block^T @ blockdiag(D) (bf16).  Y gets
       +sqrt8*L DC bias on eviction (restores +LEVEL).
  P4 : H-IDCT (+v-upsample+color for chroma via PU); chroma rhs W-upsampled
       through a broadcast AP -> psum RGB
  out: DVE tensor_scalar clamp(0,1) psum->sbuf staging, one DMA per channel.

Software-pipelined: front half (S1: load..quantize) of image i+1 is emitted
before the back half (S2: ITP..store) of image i so the PE keeps running
while DVE quantizes, and input DMAs prefetch one image ahead.
"""
import sys
sys.path.insert(0, '/opt/trn_rl_repo')
import numpy as np
import concourse.bacc as bacc
import concourse.bass as bass
import concourse.mybir as mybir
import concourse.tile as tile
from concourse import bass_utils

N_CORES = 8
IMG_PER_CORE = 4
H = W = 512
LEVEL = np.float32(128.0 / 255.0)
SQRT8L = float(np.sqrt(8.0) * LEVEL)
C_ROUND = 12582912.0   # 1.5*2^23: (x+C)-C == round-half-even(x)
F32 = mybir.dt.float32
F32R = mybir.dt.float32r
BF16 = mybir.dt.bfloat16

RGB2YCC = np.array([[0.299, 0.587, 0.114],
                    [-0.168735892, -0.331264108, 0.5],
                    [0.5, -0.418687589, -0.081312411]], dtype=np.float32)
CB_C = np.array([0.0, -0.344136286, 1.772], dtype=np.float32)
CR_C = np.array([1.402, -0.714136286, 0.0], dtype=np.float32)


def _dct8():
    i = np.arange(8)[:, None].astype(np.float64)
    j = np.arange(8)[None, :].astype(np.float64)
    m = np.sqrt(2.0 / 8) * np.cos(np.pi * (2 * j + 1) * i / 16.0)
    m[0, :] = 1.0 / np.sqrt(8.0)
    return m.astype(np.float32)


def _blockdiag(b, reps):
    r, c = b.shape
    out = np.zeros((r * reps, c * reps), dtype=np.float32)
    for k in range(reps):
        out[k * r:(k + 1) * r, k * c:(k + 1) * c] = b
    return out


def _build_consts(quantize):
    D = _dct8()
    BD_T = _blockdiag(D.T, 16)
    BD = _blockdiag(D, 16)
    pf8 = np.zeros((16, 8), dtype=np.float32)
    for ii in range(8):
        for dh in range(2):
            pf8[2 * ii + dh, :] = D[:, ii] * 0.5
    PF = _blockdiag(pf8, 8)                # [128, 64]
    pu8 = np.zeros((8, 16), dtype=np.float32)
    for jj in range(8):
        for dw in range(2):
            pu8[:, 2 * jj + dw] = D[:, jj]
    PU = _blockdiag(pu8, 8)                # [64, 128]

    consts = {}
    for c in range(3):
        consts[f"w1y{c}"] = RGB2YCC[0, c] * BD_T
        consts[f"w1c{c}"] = np.concatenate(
            [RGB2YCC[1, c] * PF, RGB2YCC[2, c] * PF], axis=1)  # [128,128]
    consts["ident"] = np.eye(128, dtype=np.float32)
    consts["w2y"] = BD_T
    consts["w2c"] = 0.5 * BD_T             # W-pool avg fold
    consts["w4y"] = BD
    for name, cb, cr in (("R", CB_C[0], CR_C[0]), ("G", CB_C[1], CR_C[1]),
                         ("B", CB_C[2], CR_C[2])):
        m = np.zeros((128, 128), dtype=np.float32)
        m[0:64, :] = cb * PU
        m[64:128, :] = cr * PU
        consts[f"w4c{name}"] = m

    q = (np.round(quantize[0].astype(np.float32) * np.float32(255.0))
         / np.float32(255.0)).astype(np.float32)
    rq = (1.0 / q.astype(np.float64)).astype(np.float32)
    consts["rqt"] = np.tile(rq.T, (16, 64)).astype(np.float32)   # [128,512]
    consts["qt"] = np.tile(q.T, (16, 64)).astype(np.float32)
    bd1 = np.zeros((128, 1), dtype=np.float32)
    bd1[0::8, 0] = -SQRT8L
    consts["bias_d1"] = bd1
    bt2 = np.zeros((128, 1), dtype=np.float32)
    bt2[0::8, 0] = SQRT8L
    consts["bias_t2"] = bt2

    cy = np.concatenate([consts[n] for n in _CY_NAMES], axis=1)
    cc = np.concatenate([consts[n] for n in _CC_NAMES], axis=1)
    late = np.concatenate([consts[n] for n in _LATE_NAMES], axis=1)
    return {"cpack_y": cy, "cpack_c": cc, "cpack_late": late}


_CONST_SHAPES = None
# packed const column layout: name -> (ncols, f32-view?)
_CY_NAMES = ("w1y0", "w1y1", "w1y2", "bias_d1")
_CC_NAMES = ("w1c0", "w1c1", "w1c2")
_LATE_NAMES = ("ident", "w2y", "w2c", "w4y", "w4cR", "w4cG", "w4cB",
               "rqt", "qt", "bias_t2")
_F32_VIEW = {"rqt", "qt", "bias_d1", "bias_t2"}


def _build_nc():
    nc = bacc.Bacc("TRN2", target_bir_lowering=False, debug=False,
                   enable_asserts=False, num_devices=N_CORES)
    x_d = nc.dram_tensor("x", [IMG_PER_CORE, 3, H, W], F32R,
                         kind="ExternalInput").ap()
    out_d = nc.dram_tensor("out", [IMG_PER_CORE, 3, H, W], F32,
                           kind="ExternalOutput").ap()
    bdw_bf_d = nc.dram_tensor("bdw_bf", [128, 128], BF16,
                              kind="ExternalInput").ap()
    cy_d = nc.dram_tensor("cpack_y", list(_CONST_SHAPES["cpack_y"]),
                          F32R, kind="ExternalInput").ap()
    cc_d = nc.dram_tensor("cpack_c", list(_CONST_SHAPES["cpack_c"]),
                          F32R, kind="ExternalInput").ap()
    cl_d = nc.dram_tensor("cpack_late", list(_CONST_SHAPES["cpack_late"]),
                          F32R, kind="ExternalInput").ap()

    ACT = mybir.ActivationFunctionType
    OP = mybir.AluOpType

    with tile.TileContext(nc) as tc:
        with tc.tile_pool(name="consts", bufs=1) as cp, \
             tc.tile_pool(name="xin", bufs=6) as xp, \
             tc.tile_pool(name="work", bufs=8) as wp, \
             tc.tile_pool(name="og", bufs=6) as ogp, \
             tc.tile_pool(name="psA", bufs=2, space="PSUM") as pA, \
             tc.tile_pool(name="psB", bufs=2, space="PSUM") as pB, \
             tc.tile_pool(name="psC", bufs=1, space="PSUM") as pC, \
             tc.tile_pool(name="psE", bufs=3, space="PSUM") as pE:

            cy_t = cp.tile(list(_CONST_SHAPES["cpack_y"]), F32R,
                           tag="c_cy", name="c_cy")
            nc.sync.dma_start(cy_t[:], cy_d)
            cc_t = cp.tile(list(_CONST_SHAPES["cpack_c"]), F32R,
                           tag="c_cc", name="c_cc")
            nc.sync.dma_start(cc_t[:], cc_d)

            # img0: fine-grained t-major loads (gpsimd queue, parallel with
            # the const triggers on sync) so P1 can start early
            X0 = {}
            for t in range(4):
                for c in range(3):
                    xt = xp.tile([128, 512], F32R, tag="x0",
                                 name=f"x0_{c}_{t}", bufs=12)
                    nc.gpsimd.dma_start(
                        xt[:], x_d[0, c, 128 * t:128 * (t + 1), :])
                    X0[c, t] = xt

            cl_t = cp.tile(list(_CONST_SHAPES["cpack_late"]), F32R,
                           tag="c_late", name="c_late")
            nc.sync.dma_start(cl_t[:], cl_d)
            bdw_bf = cp.tile([128, 128], BF16, tag="c_bdwb", name="c_bdwb")
            nc.sync.dma_start(bdw_bf[:], bdw_bf_d)

            cs = {}
            col = 0
            for n in _CY_NAMES:
                w = _CONST_SHAPES[n + "__w"]
                ap = cy_t[:, col:col + w]
                cs[n] = ap.bitcast(F32) if n in _F32_VIEW else ap
                col += w
            col = 0
            for n in _CC_NAMES:
                w = _CONST_SHAPES[n + "__w"]
                ap = cc_t[:, col:col + w]
                cs[n] = ap.bitcast(F32) if n in _F32_VIEW else ap
                col += w
            col = 0
            for n in _LATE_NAMES:
                w = _CONST_SHAPES[n + "__w"]
                ap = cl_t[:, col:col + w]
                cs[n] = ap.bitcast(F32) if n in _F32_VIEW else ap
                col += w

            def xslice(img, c, t):
                if img == 0:
                    return X0[c, t][:]
                return XB[img][c][:, 512 * t:512 * (t + 1)]

            XB = {}

            def emit_S1(img):
                """load + P1 + T1 + P2 + quantize; returns dec tiles."""
                if img > 0:
                    Xc = []
                    for c in range(3):
                        xt = xp.tile([128, 2048], F32R, tag="x",
                                     name=f"x_{img}_{c}", bufs=6)
                        src = x_d[img, c].rearrange("(t p) w -> p t w", t=4)
                        nc.sync.dma_start(xt[:], src)
                        Xc.append(xt)
                    XB[img] = Xc

                d1y, d1c = [], []
                for t in range(4):
                    psY = pA.tile([128, 512], F32, tag="p1", name="psY_t")
                    for c in range(3):
                        nc.tensor.matmul(psY[:], cs[f"w1y{c}"],
                                         xslice(img, c, t),
                                         start=(c == 0), stop=(c == 2))
                    ty = wp.tile([128, 512], F32R, tag="d1y",
                                 name=f"d1y_{img}_{t}", bufs=5)
                    nc.scalar.activation(ty[:], psY[:], ACT.Identity,
                                         bias=cs["bias_d1"])
                    d1y.append(ty)

                    psC = pA.tile([128, 256], F32, tag="p1", name="psC_t")
                    for c in range(3):
                        xs = xslice(img, c, t)
                        nc.tensor.matmul(psC[:], cs[f"w1c{c}"],
                                         xs[:, 0::2], start=(c == 0),
                                         stop=False)
                        nc.tensor.matmul(psC[:], cs[f"w1c{c}"],
                                         xs[:, 1::2], start=False,
                                         stop=(c == 2))
                    tcc = wp.tile([128, 256], F32R, tag="d1c",
                                  name=f"d1c_{img}_{t}", bufs=5)
                    nc.scalar.activation(tcc[:], psC[:], ACT.Copy)
                    d1c.append(tcc)

                t1y, t1c = [], []
                for s in range(4):
                    pty = pB.tile([128, 512], F32R, tag="tp", name="pstp_t")
                    for t in range(4):
                        nc.tensor.transpose(
                            pty[:, 128 * t:128 * (t + 1)],
                            d1y[t][:, 128 * s:128 * (s + 1)], cs["ident"])
                    sy = wp.tile([128, 512], F32R, tag="t1y",
                                 name=f"t1y_{img}_{s}", bufs=4)
                    nc.scalar.activation(sy[:], pty[:], ACT.Copy)
                    t1y.append(sy)
                for s in range(2):
                    ptc = pB.tile([128, 512], F32R, tag="tp", name="pstpc_t")
                    for t in range(4):
                        nc.tensor.transpose(
                            ptc[:, 128 * t:128 * (t + 1)],
                            d1c[t][:, 128 * s:128 * (s + 1)], cs["ident"])
                    sc = wp.tile([128, 512], F32R, tag="t1c",
                                 name=f"t1c_{img}_{s}", bufs=4)
                    nc.scalar.activation(sc[:], ptc[:], ACT.Copy)
                    t1c.append(sc)

                decy, decc = [], []
                for s in range(6):
                    ps = pC.tile([128, 512], F32, tag="mm2", name="ps2_t")
                    if s < 4:
                        nc.tensor.matmul(ps[:], cs["w2y"], t1y[s][:],
                                         start=True, stop=True)
                    else:
                        nc.tensor.matmul(ps[:], cs["w2c"], t1c[s - 4][:],
                                         start=True, stop=True)
                    ey = wp.tile([128, 512], F32, tag="ey",
                                 name=f"ey_{img}_{s}", bufs=3)
                    nc.vector.tensor_tensor(ey[:], ps[:], cs["rqt"], OP.mult)
                    nc.vector.tensor_scalar(ey[:], ey[:], C_ROUND, C_ROUND,
                                            OP.add, OP.subtract)
                    dt_ = wp.tile([128, 512], BF16, tag="dec",
                                  name=f"dec_{img}_{s}", bufs=12)
                    nc.vector.tensor_tensor(dt_[:], ey[:], cs["qt"], OP.mult)
                    (decy if s < 4 else decc).append(dt_)
                return decy, decc

            def emit_S2(img, decy, decc):
                """ITP + P4 + clamp + store."""
                t2y, t2c = [], []
                for t in range(4):
                    pt = pB.tile([128, 512], F32, tag="tp", name="psit_t")
                    for s in range(4):
                        nc.tensor.matmul(pt[:, 128 * s:128 * (s + 1)],
                                         decy[s][:, 128 * t:128 * (t + 1)],
                                         bdw_bf[:], start=True, stop=True)
                    sy = wp.tile([128, 512], F32R, tag="t2y",
                                 name=f"t2y_{img}_{t}", bufs=8)
                    nc.scalar.activation(sy[:], pt[:], ACT.Identity,
                                         bias=cs["bias_t2"])
                    t2y.append(sy)
                for t in range(4):
                    pt = pB.tile([128, 512], F32, tag="tp", name="psitc_t")
                    for s in range(2):
                        nc.tensor.matmul(pt[:, 128 * s:128 * (s + 1)],
                                         decc[s][:, 128 * t:128 * (t + 1)],
                                         bdw_bf[:], start=True, stop=True)
                    sc = wp.tile([128, 256], F32R, tag="t2c",
                                 name=f"t2c_{img}_{t}", bufs=8)
                    nc.scalar.activation(sc[:], pt[:, 0:256], ACT.Copy)
                    t2c.append(sc)

                last = img == IMG_PER_CORE - 1
                og = []
                for ci in range(3):
                    o = ogp.tile([128, 2048], F32, tag="og",
                                 name=f"og_{img}_{ci}")
                    og.append(o)
                t2c_ups = [t2c[t][:].unsqueeze(2).broadcast_to([128, 256, 2])
                           for t in range(4)]
                if last:
                    # channel-major so each channel's batched store overlaps
                    # the next channel's P4 (all t2 tiles are already final)
                    order = [(t, ci) for ci in range(3) for t in range(4)]
                else:
                    order = [(t, ci) for t in range(4) for ci in range(3)]
                # stores spread across engine DMA queues so the tail
                # transfers don't serialize on the sync queue
                last_eng = {(2, 0): nc.sync, (2, 1): nc.gpsimd,
                            (2, 2): nc.scalar, (2, 3): nc.gpsimd}
                for t, ci in order:
                    cname = "RGB"[ci]
                    ps = pE.tile([128, 512], F32, tag="mm4", name="ps4_t")
                    nc.tensor.matmul(ps[:], cs["w4y"], t2y[t][:],
                                     start=True, stop=False)
                    nc.tensor.matmul(ps[:], cs[f"w4c{cname}"], t2c_ups[t],
                                     start=False, stop=True)
                    nc.vector.tensor_scalar(
                        og[ci][:, 512 * t:512 * (t + 1)], ps[:],
                        0.0, 1.0, OP.max, OP.min)
                    if last and ci == 2:
                        last_eng[(ci, t)].dma_start(
                            out_d[img, ci, 128 * t:128 * (t + 1), :],
                            og[ci][:, 512 * t:512 * (t + 1)])
                    elif last and t == 3:
                        eng = nc.sync if ci == 0 else nc.scalar
                        dst = out_d[img, ci].rearrange("(t p) w -> p t w", t=4)
                        eng.dma_start(dst, og[ci][:])
                if not last:
                    eng = nc.gpsimd if img == IMG_PER_CORE - 2 else nc.sync
                    for ci in range(3):
                        dst = out_d[img, ci].rearrange("(t p) w -> p t w", t=4)
                        eng.dma_start(dst, og[ci][:])

            # software pipeline: S1(0) S1(1) S2(0) S1(2) S2(1) S1(3) S2(2) S2(3)
            dec = {}
            dec[0] = emit_S1(0)
            dec[1] = emit_S1(1)
            emit_S2(0, *dec[0])
            dec[2] = emit_S1(2)
            emit_S2(1, *dec[1])
            dec[3] = emit_S1(3)
            emit_S2(2, *dec[2])
            emit_S2(3, *dec[3])
    nc.compile()
    return nc


_NC_CACHE = None
TRACE = False
TRACE_DIR = None
LAST = None


def kernel(input, quantize):
    global _NC_CACHE, _CONST_SHAPES, LAST
    input = np.asarray(input, dtype=np.float32)
    quantize = np.asarray(quantize, dtype=np.float32)
    consts = _build_consts(quantize)
    if _CONST_SHAPES is None:
        _CONST_SHAPES = {k: v.shape for k, v in consts.items()}
        widths = {n: 128 for n in _CY_NAMES + _CC_NAMES + _LATE_NAMES}
        widths.update({"bias_d1": 1, "bias_t2": 1, "rqt": 512, "qt": 512})
        for n, w in widths.items():
            _CONST_SHAPES[n + "__w"] = w
    if _NC_CACHE is None:
        _NC_CACHE = _build_nc()
    nc = _NC_CACHE

    import ml_dtypes
    bdw_bf = _blockdiag(_dct8(), 16).astype(ml_dtypes.bfloat16)

    in_maps = []
    for core in range(N_CORES):
        shard = np.ascontiguousarray(
            input[core * IMG_PER_CORE:(core + 1) * IMG_PER_CORE])
        m = {"x": shard, "bdw_bf": bdw_bf}
        m.update(consts)
        in_maps.append(m)
    kw = {}
    if TRACE:
        kw = dict(trace=True, tmpdir=TRACE_DIR)
    res = bass_utils.run_bass_kernel_spmd(nc, in_maps,
                                          core_ids=list(range(N_CORES)), **kw)
    LAST = res
    out = np.concatenate([res.results[i]["out"] for i in range(N_CORES)],
                         axis=0)
    return out.astype(np.float32)


# revision 23
# speedup vs baseline: 1.0313x; 1.0313x over previous
"""JPEG layer (nn_JpegLayer) Trainium2 Bass kernel, 8-core data parallel.

Pipeline per image (per core: 4 images of [3,512,512]):
  P1 : 3-accum matmuls fold RGB->YCC color mix + H-DCT; chroma additionally
       W-pools via paired stride-2 rhs matmuls (6 accums at N=256).  Y level
       shift (-sqrt8*L at DC rows) folds into the eviction bias (scalar eng).
  T1 : PE transposes -> [w, h'freq]
  P2 : W-DCT (0.5x pool fold for chroma) -> full 2D coeffs psum
  Q  : ey = d*(1/q) (DVE); round via +/-1.5*2^23 (DVE); dec = r*q -> bf16
  ITP: fused W-IDCT + transpose as plain matmuls with the dec block as
       stationary: psum[., s] = dec_block^T @ blockdiag(D) (bf16).  Y gets
       +sqrt8*L DC bias on eviction (restores +LEVEL).
  P4 : H-IDCT (+v-upsample+color for chroma via PU); chroma rhs W-upsampled
       through a broadcast AP -> psum RGB
  out: DVE tensor_scalar clamp(0,1) psum->sbuf staging, one DMA per channel.

Software-pipelined: front half (S1: load..quantize) of image i+1 is emitted
before the back half (S2: ITP..store) of image i so the PE keeps running
while DVE quantizes, and input DMAs prefetch one image ahead.
"""
import sys
sys.path.insert(0, '/opt/trn_rl_repo')
import numpy as np
import concourse.bacc as bacc
import concourse.bass as bass
import concourse.mybir as mybir
import concourse.tile as tile
from concourse import bass_utils

N_CORES = 8
IMG_PER_CORE = 4
H = W = 512
LEVEL = np.float32(128.0 / 255.0)
SQRT8L = float(np.sqrt(8.0) * LEVEL)
C_ROUND = 12582912.0   # 1.5*2^23: (x+C)-C == round-half-even(x)
F32 = mybir.dt.float32
F32R = mybir.dt.float32r
BF16 = mybir.dt.bfloat16

RGB2YCC = np.array([[0.299, 0.587, 0.114],
                    [-0.168735892, -0.331264108, 0.5],
                    [0.5, -0.418687589, -0.081312411]], dtype=np.float32)
CB_C = np.array([0.0, -0.344136286, 1.772], dtype=np.float32)
CR_C = np.array([1.402, -0.714136286, 0.0], dtype=np.float32)


def _dct8():
    i = np.arange(8)[:, None].astype(np.float64)
    j = np.arange(8)[None, :].astype(np.float64)
    m = np.sqrt(2.0 / 8) * np.cos(np.pi * (2 * j + 1) * i / 16.0)
    m[0, :] = 1.0 / np.sqrt(8.0)
    return m.astype(np.float32)


def _blockdiag(b, reps):
    r, c = b.shape
    out = np.zeros((r * reps, c * reps), dtype=np.float32)
    for k in range(reps):
        out[k * r:(k + 1) * r, k * c:(k + 1) * c] = b
    return out


def _build_consts(quantize):
    D = _dct8()
    BD_T = _blockdiag(D.T, 16)
    BD = _blockdiag(D, 16)
    pf8 = np.zeros((16, 8), dtype=np.float32)
    for ii in range(8):
        for dh in range(2):
            pf8[2 * ii + dh, :] = D[:, ii] * 0.5
    PF = _blockdiag(pf8, 8)                # [128, 64]
    pu8 = np.zeros((8, 16), dtype=np.float32)
    for jj in range(8):
        for dw in range(2):
            pu8[:, 2 * jj + dw] = D[:, jj]
    PU = _blockdiag(pu8, 8)                # [64, 128]

    consts = {}
    for c in range(3):
        consts[f"w1y{c}"] = RGB2YCC[0, c] * BD_T
        consts[f"w1c{c}"] = np.concatenate(
            [RGB2YCC[1, c] * PF, RGB2YCC[2, c] * PF], axis=1)  # [128,128]
    consts["ident"] = np.eye(128, dtype=np.float32)
    consts["w2y"] = BD_T
    consts["w2c"] = 0.5 * BD_T             # W-pool avg fold
    consts["w4y"] = BD
    for name, cb, cr in (("R", CB_C[0], CR_C[0]), ("G", CB_C[1], CR_C[1]),
                         ("B", CB_C[2], CR_C[2])):
        m = np.zeros((128, 128), dtype=np.float32)
        m[0:64, :] = cb * PU
        m[64:128, :] = cr * PU
        consts[f"w4c{name}"] = m

    q = (np.round(quantize[0].astype(np.float32) * np.float32(255.0))
         / np.float32(255.0)).astype(np.float32)
    rq = (1.0 / q.astype(np.float64)).astype(np.float32)
    consts["rqt"] = np.tile(rq.T, (16, 64)).astype(np.float32)   # [128,512]
    consts["qt"] = np.tile(q.T, (16, 64)).astype(np.float32)
    bd1 = np.zeros((128, 1), dtype=np.float32)
    bd1[0::8, 0] = -SQRT8L
    consts["bias_d1"] = bd1
    bt2 = np.zeros((128, 1), dtype=np.float32)
    bt2[0::8, 0] = SQRT8L
    consts["bias_t2"] = bt2

    cy = np.concatenate([consts[n] for n in _CY_NAMES], axis=1)
    cc = np.concatenate([consts[n] for n in _CC_NAMES], axis=1)
    late = np.concatenate([consts[n] for n in _LATE_NAMES], axis=1)
    return {"cpack_y": cy, "cpack_c": cc, "cpack_late": late}


_CONST_SHAPES = None
# packed const column layout: name -> (ncols, f32-view?)
_CY_NAMES = ("w1y0", "w1y1", "w1y2", "bias_d1")
_CC_NAMES = ("w1c0", "w1c1", "w1c2")
_LATE_NAMES = ("ident", "w2y", "w2c", "w4y", "w4cR", "w4cG", "w4cB",
               "rqt", "qt", "bias_t2")
_F32_VIEW = {"rqt", "qt", "bias_d1", "bias_t2"}


def _build_nc():
    nc = bacc.Bacc("TRN2", target_bir_lowering=False, debug=False,
                   enable_asserts=False, num_devices=N_CORES)
    x_d = nc.dram_tensor("x", [IMG_PER_CORE, 3, H, W], F32R,
                         kind="ExternalInput").ap()
    out_d = nc.dram_tensor("out", [IMG_PER_CORE, 3, H, W], F32,
                           kind="ExternalOutput").ap()
    bdw_bf_d = nc.dram_tensor("bdw_bf", [128, 128], BF16,
                              kind="ExternalInput").ap()
    cy_d = nc.dram_tensor("cpack_y", list(_CONST_SHAPES["cpack_y"]),
                          F32R, kind="ExternalInput").ap()
    cc_d = nc.dram_tensor("cpack_c", list(_CONST_SHAPES["cpack_c"]),
                          F32R, kind="ExternalInput").ap()
    cl_d = nc.dram_tensor("cpack_late", list(_CONST_SHAPES["cpack_late"]),
                          F32R, kind="ExternalInput").ap()

    ACT = mybir.ActivationFunctionType
    OP = mybir.AluOpType

    with tile.TileContext(nc) as tc:
        with tc.tile_pool(name="consts", bufs=1) as cp, \
             tc.tile_pool(name="xin", bufs=6) as xp, \
             tc.tile_pool(name="work", bufs=8) as wp, \
             tc.tile_pool(name="og", bufs=6) as ogp, \
             tc.tile_pool(name="psA", bufs=2, space="PSUM") as pA, \
             tc.tile_pool(name="psB", bufs=2, space="PSUM") as pB, \
             tc.tile_pool(name="psC", bufs=1, space="PSUM") as pC, \
             tc.tile_pool(name="psE", bufs=3, space="PSUM") as pE:

            cy_t = cp.tile(list(_CONST_SHAPES["cpack_y"]), F32R,
                           tag="c_cy", name="c_cy")
            nc.sync.dma_start(cy_t[:], cy_d)
            cc_t = cp.tile(list(_CONST_SHAPES["cpack_c"]), F32R,
                           tag="c_cc", name="c_cc")
            nc.sync.dma_start(cc_t[:], cc_d)

            # img0: fine-grained t-major loads, triggers split between the
            # gpsimd and sync queues so P1 can start early and stay fed
            X0 = {}
            for t in range(4):
                for c in range(3):
                    xt = xp.tile([128, 512], F32R, tag="x0",
                                 name=f"x0_{c}_{t}", bufs=12)
                    eng = nc.gpsimd if (t * 3 + c) % 2 == 0 else nc.sync
                    eng.dma_start(
                        xt[:], x_d[0, c, 128 * t:128 * (t + 1), :])
                    X0[c, t] = xt

            cl_t = cp.tile(list(_CONST_SHAPES["cpack_late"]), F32R,
                           tag="c_late", name="c_late")
            nc.sync.dma_start(cl_t[:], cl_d)
            bdw_bf = cp.tile([128, 128], BF16, tag="c_bdwb", name="c_bdwb")
            nc.sync.dma_start(bdw_bf[:], bdw_bf_d)

            cs = {}
            col = 0
            for n in _CY_NAMES:
                w = _CONST_SHAPES[n + "__w"]
                ap = cy_t[:, col:col + w]
                cs[n] = ap.bitcast(F32) if n in _F32_VIEW else ap
                col += w
            col = 0
            for n in _CC_NAMES:
                w = _CONST_SHAPES[n + "__w"]
                ap = cc_t[:, col:col + w]
                cs[n] = ap.bitcast(F32) if n in _F32_VIEW else ap
                col += w
            col = 0
            for n in _LATE_NAMES:
                w = _CONST_SHAPES[n + "__w"]
                ap = cl_t[:, col:col + w]
                cs[n] = ap.bitcast(F32) if n in _F32_VIEW else ap
                col += w

            def xslice(img, c, t):
                if img == 0:
                    return X0[c, t][:]
                return XB[img][c][:, 512 * t:512 * (t + 1)]

            XB = {}

            def emit_S1(img):
                """load + P1 + T1 + P2 + quantize; returns dec tiles."""
                if img > 0:
                    Xc = []
                    for c in range(3):
                        xt = xp.tile([128, 2048], F32R, tag="x",
                                     name=f"x_{img}_{c}", bufs=6)
                        src = x_d[img, c].rearrange("(t p) w -> p t w", t=4)
                        nc.sync.dma_start(xt[:], src)
                        Xc.append(xt)
                    XB[img] = Xc

                d1y, d1c = [], []
                for t in range(4):
                    psY = pA.tile([128, 512], F32, tag="p1", name="psY_t")
                    for c in range(3):
                        nc.tensor.matmul(psY[:], cs[f"w1y{c}"],
                                         xslice(img, c, t),
                                         start=(c == 0), stop=(c == 2))
                    ty = wp.tile([128, 512], F32R, tag="d1y",
                                 name=f"d1y_{img}_{t}", bufs=5)
                    nc.scalar.activation(ty[:], psY[:], ACT.Identity,
                                         bias=cs["bias_d1"])
                    d1y.append(ty)

                    psC = pA.tile([128, 256], F32, tag="p1", name="psC_t")
                    for c in range(3):
                        xs = xslice(img, c, t)
                        nc.tensor.matmul(psC[:], cs[f"w1c{c}"],
                                         xs[:, 0::2], start=(c == 0),
                                         stop=False)
                        nc.tensor.matmul(psC[:], cs[f"w1c{c}"],
                                         xs[:, 1::2], start=False,
                                         stop=(c == 2))
                    tcc = wp.tile([128, 256], F32R, tag="d1c",
                                  name=f"d1c_{img}_{t}", bufs=5)
                    nc.scalar.activation(tcc[:], psC[:], ACT.Copy)
                    d1c.append(tcc)

                t1y, t1c = [], []
                for s in range(4):
                    pty = pB.tile([128, 512], F32R, tag="tp", name="pstp_t")
                    for t in range(4):
                        nc.tensor.transpose(
                            pty[:, 128 * t:128 * (t + 1)],
                            d1y[t][:, 128 * s:128 * (s + 1)], cs["ident"])
                    sy = wp.tile([128, 512], F32R, tag="t1y",
                                 name=f"t1y_{img}_{s}", bufs=4)
                    nc.scalar.activation(sy[:], pty[:], ACT.Copy)
                    t1y.append(sy)
                for s in range(2):
                    ptc = pB.tile([128, 512], F32R, tag="tp", name="pstpc_t")
                    for t in range(4):
                        nc.tensor.transpose(
                            ptc[:, 128 * t:128 * (t + 1)],
                            d1c[t][:, 128 * s:128 * (s + 1)], cs["ident"])
                    sc = wp.tile([128, 512], F32R, tag="t1c",
                                 name=f"t1c_{img}_{s}", bufs=4)
                    nc.scalar.activation(sc[:], ptc[:], ACT.Copy)
                    t1c.append(sc)

                decy, decc = [], []
                for s in range(6):
                    ps = pC.tile([128, 512], F32, tag="mm2", name="ps2_t")
                    if s < 4:
                        nc.tensor.matmul(ps[:], cs["w2y"], t1y[s][:],
                                         start=True, stop=True)
                    else:
                        nc.tensor.matmul(ps[:], cs["w2c"], t1c[s - 4][:],
                                         start=True, stop=True)
                    ey = wp.tile([128, 512], F32, tag="ey",
                                 name=f"ey_{img}_{s}", bufs=3)
                    nc.vector.tensor_tensor(ey[:], ps[:], cs["rqt"], OP.mult)
                    nc.vector.tensor_scalar(ey[:], ey[:], C_ROUND, C_ROUND,
                                            OP.add, OP.subtract)
                    dt_ = wp.tile([128, 512], BF16, tag="dec",
                                  name=f"dec_{img}_{s}", bufs=12)
                    nc.vector.tensor_tensor(dt_[:], ey[:], cs["qt"], OP.mult)
                    (decy if s < 4 else decc).append(dt_)
                return decy, decc

            def emit_S2(img, decy, decc):
                """ITP + P4 + clamp + store."""
                t2y, t2c = [], []
                for t in range(4):
                    pt = pB.tile([128, 512], F32, tag="tp", name="psit_t")
                    for s in range(4):
                        nc.tensor.matmul(pt[:, 128 * s:128 * (s + 1)],
                                         decy[s][:, 128 * t:128 * (t + 1)],
                                         bdw_bf[:], start=True, stop=True)
                    sy = wp.tile([128, 512], F32R, tag="t2y",
                                 name=f"t2y_{img}_{t}", bufs=8)
                    nc.scalar.activation(sy[:], pt[:], ACT.Identity,
                                         bias=cs["bias_t2"])
                    t2y.append(sy)
                for t in range(4):
                    pt = pB.tile([128, 512], F32, tag="tp", name="psitc_t")
                    for s in range(2):
                        nc.tensor.matmul(pt[:, 128 * s:128 * (s + 1)],
                                         decc[s][:, 128 * t:128 * (t + 1)],
                                         bdw_bf[:], start=True, stop=True)
                    sc = wp.tile([128, 256], F32R, tag="t2c",
                                 name=f"t2c_{img}_{t}", bufs=8)
                    nc.scalar.activation(sc[:], pt[:, 0:256], ACT.Copy)
                    t2c.append(sc)

                last = img == IMG_PER_CORE - 1
                og = []
                for ci in range(3):
                    o = ogp.tile([128, 2048], F32, tag="og",
                                 name=f"og_{img}_{ci}")
                    og.append(o)
                t2c_ups = [t2c[t][:].unsqueeze(2).broadcast_to([128, 256, 2])
                           for t in range(4)]
                if last:
                    # channel-major so each channel's batched store overlaps
                    # the next channel's P4 (all t2 tiles are already final)
                    order = [(t, ci) for ci in range(3) for t in range(4)]
                else:
                    order = [(t, ci) for t in range(4) for ci in range(3)]
                # stores spread across engine DMA queues so the tail
                # transfers don't serialize on the sync queue
                engs = (nc.sync, nc.scalar, nc.gpsimd)
                for k, (t, ci) in enumerate(order):
                    cname = "RGB"[ci]
                    ps = pE.tile([128, 512], F32, tag="mm4", name="ps4_t")
                    nc.tensor.matmul(ps[:], cs["w4y"], t2y[t][:],
                                     start=True, stop=False)
                    nc.tensor.matmul(ps[:], cs[f"w4c{cname}"], t2c_ups[t],
                                     start=False, stop=True)
                    nc.vector.tensor_scalar(
                        og[ci][:, 512 * t:512 * (t + 1)], ps[:],
                        0.0, 1.0, OP.max, OP.min)
                    if last:
                        engs[k % 3].dma_start(
                            out_d[img, ci, 128 * t:128 * (t + 1), :],
                            og[ci][:, 512 * t:512 * (t + 1)])
                if not last:
                    eng = nc.gpsimd if img == IMG_PER_CORE - 2 else nc.sync
                    for ci in range(3):
                        dst = out_d[img, ci].rearrange("(t p) w -> p t w", t=4)
                        eng.dma_start(dst, og[ci][:])

            # software pipeline: S1(0) S1(1) S2(0) S1(2) S2(1) S1(3) S2(2) S2(3)
            dec = {}
            dec[0] = emit_S1(0)
            dec[1] = emit_S1(1)
            emit_S2(0, *dec[0])
            dec[2] = emit_S1(2)
            emit_S2(1, *dec[1])
            dec[3] = emit_S1(3)
            emit_S2(2, *dec[2])
            emit_S2(3, *dec[3])
    nc.compile()
    return nc


_NC_CACHE = None
TRACE = False
TRACE_DIR = None
LAST = None


def kernel(input, quantize):
    global _NC_CACHE, _CONST_SHAPES, LAST
    input = np.asarray(input, dtype=np.float32)
    quantize = np.asarray(quantize, dtype=np.float32)
    consts = _build_consts(quantize)
    if _CONST_SHAPES is None:
        _CONST_SHAPES = {k: v.shape for k, v in consts.items()}
        widths = {n: 128 for n in _CY_NAMES + _CC_NAMES + _LATE_NAMES}
        widths.update({"bias_d1": 1, "bias_t2": 1, "rqt": 512, "qt": 512})
        for n, w in widths.items():
            _CONST_SHAPES[n + "__w"] = w
    if _NC_CACHE is None:
        _NC_CACHE = _build_nc()
    nc = _NC_CACHE

    import ml_dtypes
    bdw_bf = _blockdiag(_dct8(), 16).astype(ml_dtypes.bfloat16)

    in_maps = []
    for core in range(N_CORES):
        shard = np.ascontiguousarray(
            input[core * IMG_PER_CORE:(core + 1) * IMG_PER_CORE])
        m = {"x": shard, "bdw_bf": bdw_bf}
        m.update(consts)
        in_maps.append(m)
    kw = {}
    if TRACE:
        kw = dict(trace=True, tmpdir=TRACE_DIR)
    res = bass_utils.run_bass_kernel_spmd(nc, in_maps,
                                          core_ids=list(range(N_CORES)), **kw)
    LAST = res
    out = np.concatenate([res.results[i]["out"] for i in range(N_CORES)],
                         axis=0)
    return out.astype(np.float32)
